# revision 1
# baseline (speedup 1.0000x reference)
"""Block-sparse attention TRN2 kernel (8 NeuronCores, SPMD over batch*heads).

Contract: kernel(**inputs) takes FULL unsharded inputs
  query/key/value: (2, 16, 2048, 128) f32, block_mask: (16, 16) bool,
  block_size: 128
and returns the FULL (2, 16, 2048, 128) f32 output.

Math per (b, h): for each 128x128 block pair (i, j) with block_mask[i, j]:
  A_ij = softmax(Q_i K_j^T / sqrt(128)) (softmax per block row, no
  cross-block merge), O_i = sum_j A_ij V_j.

Device layout ([k, q] orientation so no on-chip transposes are needed):
  For key block j, scores for the active query blocks are packed into
  512-col (one PSUM bank) chunks: S^T = matmul(lhsT=KT[:, j], rhs=QT runs)
  in fp32r (full-rate fp32). exp on ACT (PSUM f32 -> SBUF f16).
  Denominators = column sums via matmul(lhsT=ones[128,128]), replicated
  across partitions in PSUM. reciprocal_approx_fast (DVE), then
  Ahat = E * r elementwise (alternating DVE / GPSIMD). O^T += V_j^T.T @ Ahat^T accumulates in PSUM over j; drained
  via ACT copy + DMA per bank. Q^T/K^T/V packing and the final
  O^T -> O transpose happen on the host.
"""

import math

import numpy as np

B, H, S, D = 2, 16, 2048, 128
BS = 128
NB = S // BS
N_CORES = 8
N_HEADS = B * H
HPC = N_HEADS // N_CORES  # heads per core
CH = 512  # chunk columns = one PSUM bank of f32
SCALE = 1.0 / math.sqrt(float(D))


def _plan(mask):
    """Mask-derived emission plan (shared by every head/core).

    Returns a flat chunk schedule; each chunk is (used, mm1s, pieces) with
      mm1s   = (off_in_chunk, [qoff, ...], width, j); two qoffs means a
               paired single-block matmul via a 3-level access pattern.
      pieces = (q_out_col, width, off_in_chunk, j) MM2 pieces, split at
               output PSUM bank boundaries and first-touch flips.
    Partial tail chunks are merged ACROSS key blocks j (exp/denominator/
    normalize are j-agnostic; all KT/V slices are SBUF-resident), which
    cuts per-chunk op overheads on ACT/DVE by ~20%.
    """
    mask = np.asarray(mask).astype(bool)
    assert mask.shape == (NB, NB)
    cap = CH // BS  # blocks per chunk

    # Per-j FFD bin packing into <=cap-block bins.
    groups = []  # (j, [(i0, ln), ...]) per finalized bin, emission order
    pending = []  # [(j, item)] accumulating partial tails
    pend_fill = 0

    def flush():
        nonlocal pend_fill
        if pending:
            groups.append(list(pending))
            pending.clear()
            pend_fill = 0

    for j in range(NB):
        act = [i for i in range(NB) if mask[i, j]]
        runs = []
        for i in act:
            if runs and runs[-1][0] + runs[-1][1] == i:
                runs[-1][1] += 1
            else:
                runs.append([i, 1])
        items = []
        for i0, ln in runs:
            while ln > cap:
                items.append((i0, cap))
                i0 += cap
                ln -= cap
            items.append((i0, ln))
        bins = []
        for i0, ln in sorted(items, key=lambda x: -x[1]):
            for b in bins:
                if b[0] + ln <= cap:
                    b[0] += ln
                    b[1].append((i0, ln))
                    break
            else:
                bins.append([ln, [(i0, ln)]])
        for fill, bitems in bins:
            if fill == cap:
                groups.append([(j, it) for it in sorted(bitems)])
            else:
                if pend_fill + fill > cap:
                    flush()
                pending.extend((j, it) for it in sorted(bitems))
                pend_fill += fill
    flush()

    # Lay out each chunk and derive matmul descriptors + output pieces in
    # emission order (first-touch of an output block = overwrite; later
    # touches accumulate; a single matmul must be uniformly one or the
    # other and may not straddle an output bank).
    sched = []
    bank_counts = [0] * (S // CH)
    touched = set()
    for gitems in groups:
        byj = {}
        for j, it in gitems:
            byj.setdefault(j, []).append(it)
        placed = []  # (off, qoff, w, j)
        mm1s = []
        off = 0
        for j in sorted(byj):
            jitems = byj[j]
            longs = sorted([it for it in jitems if it[1] > 1])
            singles = sorted([it for it in jitems if it[1] == 1])
            sing_offs = []
            for i0, ln in longs + singles:
                placed.append((off, i0 * BS, ln * BS, j))
                if ln > 1:
                    mm1s.append((off, [i0 * BS], ln * BS, j))
                else:
                    sing_offs.append((off, i0 * BS))
                off += ln * BS
            for k in range(0, len(sing_offs) - 1, 2):
                mm1s.append(
                    (sing_offs[k][0], [sing_offs[k][1], sing_offs[k + 1][1]],
                     2 * BS, j)
                )
            if len(sing_offs) % 2:
                mm1s.append((sing_offs[-1][0], [sing_offs[-1][1]], BS, j))
        used = off
        pieces = []
        for o, qoff, w, j in placed:
            ib0 = qoff // BS
            nblk = w // BS
            blk = 0
            while blk < nblk:
                ib = ib0 + blk
                ft = ib not in touched
                bank = (ib * BS) // CH
                end = blk + 1
                while end < nblk:
                    ib2 = ib0 + end
                    if (ib2 not in touched) != ft or (ib2 * BS) // CH != bank:
                        break
                    end += 1
                for b2 in range(blk, end):
                    touched.add(ib0 + b2)
                qo = ib * BS
                wp = (end - blk) * BS
                pieces.append((qo, wp, o + (qo - qoff), j))
                bank_counts[bank] += 1
                blk = end
        sched.append((used, mm1s, pieces))
    empty_rows = [i for i in range(NB) if not mask[i].any()]
    return sched, bank_counts, empty_rows


def _build(mask):
    import concourse.bass as bass
    import concourse.bacc as bacc
    import concourse.tile as tile
    from concourse import mybir

    f32 = mybir.dt.float32
    f32r = mybir.dt.float32r
    f16 = mybir.dt.float16
    AF = mybir.ActivationFunctionType

    sched, bank_counts, empty_rows = _plan(mask)

    nc = bacc.Bacc(
        "TRN2",
        target_bir_lowering=False,
        debug=False,
        enable_asserts=False,
        num_devices=N_CORES,
    )
    qt_d = nc.dram_tensor("qt", (HPC, D, S), f32r, kind="ExternalInput").ap()
    kt_d = nc.dram_tensor("kt", (HPC, D, S), f32r, kind="ExternalInput").ap()
    v_d = nc.dram_tensor("v", (HPC, BS, NB * BS), f16, kind="ExternalInput").ap()
    ot_d = nc.dram_tensor("ot", (HPC, D, S), f32, kind="ExternalOutput").ap()

    with tile.TileContext(nc) as tc:
        with (
            tc.tile_pool(name="heads", bufs=3) as heads,
            tc.tile_pool(name="const", bufs=1) as const,
            tc.tile_pool(name="e", bufs=8) as epool,
            tc.tile_pool(name="eh", bufs=8) as ehpool,
            tc.tile_pool(name="r", bufs=8) as rpool,
            tc.tile_pool(name="outp", bufs=4) as outpool,
            tc.tile_pool(name="ps_s", bufs=2, space="PSUM") as ps_s,
            tc.tile_pool(name="ps_d", bufs=2, space="PSUM") as ps_d,
            tc.tile_pool(name="ps_o", bufs=1, space="PSUM") as ps_o,
        ):
            ones_t = const.tile([BS, BS], f16)
            nc.vector.memset(ones_t[:], 1.0)

            for h in range(HPC):
                qt_t = heads.tile([D, S], f32r, tag="qt")
                nc.sync.dma_start(out=qt_t[:], in_=qt_d[h])
                kt_t = heads.tile([D, S], f32r, tag="kt")
                nc.sync.dma_start(out=kt_t[:], in_=kt_d[h])
                v_t = heads.tile([BS, NB * BS], f16, tag="v")
                nc.sync.dma_start(out=v_t[:], in_=v_d[h])

                o_ps = ps_o.tile([D, S], f32)
                for i in empty_rows:
                    nc.vector.memset(o_ps[:, i * BS : (i + 1) * BS], 0.0)

                remaining = list(bank_counts)
                started = set()
                for cno, (used, mm1s, pieces) in enumerate(sched):
                    s_ps = ps_s.tile([BS, CH], f32)
                    for idx, (off, qoffs, w, j) in enumerate(mm1s):
                        if len(qoffs) == 2:
                            base = qt_t[:, qoffs[0] : qoffs[0] + BS]
                            rhs = bass.AP(
                                tensor=base.tensor,
                                offset=base.offset,
                                ap=[
                                    base.ap[0],
                                    [qoffs[1] - qoffs[0], 2],
                                    [1, BS],
                                ],
                            )
                        else:
                            rhs = qt_t[:, qoffs[0] : qoffs[0] + w]
                        nc.tensor.matmul(
                            s_ps[:, off : off + w],
                            lhsT=kt_t[:, j * BS : (j + 1) * BS],
                            rhs=rhs,
                            start=(idx == 0),
                            stop=(idx == len(mm1s) - 1),
                        )
                    e_t = epool.tile([BS, CH], f16)
                    nc.scalar.activation(
                        e_t[:, :used], s_ps[:, :used], AF.Exp, scale=SCALE
                    )
                    d_ps = ps_d.tile([BS, CH], f32)
                    nc.tensor.matmul(
                        d_ps[:, :used],
                        lhsT=ones_t[:],
                        rhs=e_t[:, :used],
                        start=True,
                        stop=True,
                    )
                    r_t = rpool.tile([BS, CH], f32)
                    nc.vector.reciprocal_approx_fast(r_t[:, :used], d_ps[:, :used])
                    eh_t = ehpool.tile([BS, CH], f16)
                    # Alternate the normalize multiply between DVE and the
                    # otherwise-idle GPSIMD (measured ~1.15us there vs
                    # ~0.67us on DVE; DVE is the busiest engine).
                    mult_eng = nc.gpsimd if cno % 2 == 1 else nc.vector
                    mult_eng.tensor_tensor(
                        out=eh_t[:, :used],
                        in0=e_t[:, :used],
                        in1=r_t[:, :used],
                        op=mybir.AluOpType.mult,
                    )
                    for qo, wp, op, j in pieces:
                        b = qo // CH
                        first = b not in started
                        started.add(b)
                        remaining[b] -= 1
                        nc.tensor.matmul(
                            o_ps[:, qo : qo + wp],
                            lhsT=v_t[:, j * BS : (j + 1) * BS],
                            rhs=eh_t[:, op : op + wp],
                            start=first,
                            stop=(remaining[b] == 0),
                        )
                for b in range(S // CH):
                    o_sb = outpool.tile([D, CH], f32, tag="osb")
                    nc.scalar.copy(o_sb[:], o_ps[:, b * CH : (b + 1) * CH])
                    nc.sync.dma_start(
                        out=ot_d[h, :, b * CH : (b + 1) * CH], in_=o_sb[:]
                    )

    nc.finalize()
    return nc


_CACHE = {}


def _get_program(mask):
    key = np.asarray(mask).astype(bool).tobytes()
    if key not in _CACHE:
        _CACHE[key] = _build(mask)
    return _CACHE[key]


def _shard_inputs(query, key, value):
    q = np.ascontiguousarray(query, dtype=np.float32).reshape(N_HEADS, S, D)
    k = np.ascontiguousarray(key, dtype=np.float32).reshape(N_HEADS, S, D)
    v = np.ascontiguousarray(value, dtype=np.float32).reshape(N_HEADS, S, D)
    qt = np.ascontiguousarray(q.transpose(0, 2, 1))  # (32, D, S)
    kt = np.ascontiguousarray(k.transpose(0, 2, 1))
    v16 = np.ascontiguousarray(
        v.reshape(N_HEADS, NB, BS, D).transpose(0, 2, 1, 3).astype(np.float16)
    ).reshape(N_HEADS, BS, NB * BS)
    in_maps = []
    for c in range(N_CORES):
        sl = slice(c * HPC, (c + 1) * HPC)
        in_maps.append(
            {
                "qt": np.ascontiguousarray(qt[sl]),
                "kt": np.ascontiguousarray(kt[sl]),
                "v": np.ascontiguousarray(v16[sl]),
            }
        )
    return in_maps


def _unshard_output(results):
    ot = np.concatenate([r["ot"] for r in results], axis=0)  # (32, D, S)
    out = ot.transpose(0, 2, 1).reshape(B, H, S, D)
    return np.ascontiguousarray(out, dtype=np.float32)


def kernel(query, key, value, block_mask, block_size, _trace=False):
    from concourse.bass_utils import run_bass_kernel_spmd

    assert int(block_size) == BS
    nc = _get_program(block_mask)
    in_maps = _shard_inputs(query, key, value)
    res = run_bass_kernel_spmd(nc, in_maps, core_ids=list(range(N_CORES)), trace=_trace)
    out = _unshard_output(res.results)
    if _trace:
        return out, res
    return out

